# revision 11
# baseline (speedup 1.0000x reference)
"""MultiHead GQA (16 q heads / 4 kv heads, E=1024, n=2048, b=1) on 8 TRN2 cores.

Strategy: shard the 2048 query positions across the 8 cores (256 each); every
core computes the full K/V projections for all 2048 key positions (replicating
that small matmul is far cheaper than any collective at this size), runs
attention + layernorm for its own 256 tokens, and writes its 256x1024 slice.

All heavy matmuls run in bf16 (fp32 PSUM accumulation).  The host pre-
transposes and pre-casts the operands so that every matmul contraction
dimension lands on SBUF partitions with no on-device transposes; the only
on-device transposes are tiny bf16 DMA X-bar transposes of the attention
output (64x256 per head) and of the softmax denominators.

The emission order software-pipelines PE: the attention stream is a single
sequence of (pair, s-tile) slot groups; each group's S matmuls are followed
by projection "filler" matmuls and the *previous* group's O/R matmuls, so
the (in-order) PE queue never blocks on the exp that gates O/R.

RoPE in the reference is the identity for b=1 (seq index = batch index = 0,
so cos=1 / sin=0 exactly); it is therefore omitted.
"""

import numpy as np
import ml_dtypes

import concourse.bass as bass
import concourse.bacc as bacc
import concourse.tile as tile
from concourse import mybir
from concourse import bass_utils

F32 = mybir.dt.float32
BF16 = mybir.dt.bfloat16
AF = mybir.ActivationFunctionType
ALU = mybir.AluOpType

N_CORES = 8
E = 1024
QH = 16
KVH = 4
HD = 64
KVE = KVH * HD
SEQ = 2048
T = SEQ // N_CORES   # 256 query tokens per core
ST = SEQ // 128      # 16 key s-tiles
ET = E // 128        # 8 contraction e-tiles
EPS = 1e-5
SCALE = 1.0 / (HD ** 0.5)

# Head bookkeeping: q-proj channel-tile t packs head EH[t] in partitions 0-63
# and head OH[t] in partitions 64-127.  EH heads use even kv heads (0, 2),
# OH heads use odd kv heads (1, 3), which matches the natural K-proj layout
# (K channel-tile 0 = kv0|kv1, tile 1 = kv2|kv3) with no partition shifts.
EH = [0, 1, 2, 3, 8, 9, 10, 11]
OH = [4, 5, 6, 7, 12, 13, 14, 15]
KV_LO = [EH[2 * p] // 4 for p in range(4)]   # [0, 0, 2, 2]
KV_HI = [OH[2 * p] // 4 for p in range(4)]   # [1, 1, 3, 3]

COLPERM = np.concatenate(
    [np.r_[EH[t] * HD:(EH[t] + 1) * HD, OH[t] * HD:(OH[t] + 1) * HD]
     for t in range(8)])

_CACHE = {}


def _build():
    nc = bacc.Bacc("TRN2", target_bir_lowering=False, debug=False)

    qT_d = nc.dram_tensor("qT", [E, T], BF16, kind="ExternalInput").ap()
    kT_d = nc.dram_tensor("kT", [E, SEQ], BF16, kind="ExternalInput").ap()
    vT_d = nc.dram_tensor("vT", [E, SEQ], BF16, kind="ExternalInput").ap()
    wqT_d = nc.dram_tensor("wqT", [E, E], BF16, kind="ExternalInput").ap()
    wkT_d = nc.dram_tensor("wkT", [E, KVE], BF16, kind="ExternalInput").ap()
    wvT_d = nc.dram_tensor("wvT", [E, KVE], BF16, kind="ExternalInput").ap()
    bq_d = nc.dram_tensor("bq", [128, 8], F32, kind="ExternalInput").ap()
    bk_d = nc.dram_tensor("bk", [128, 2], F32, kind="ExternalInput").ap()
    bv_d = nc.dram_tensor("bv", [KVE], F32, kind="ExternalInput").ap()
    gam_d = nc.dram_tensor("gam", [E], F32, kind="ExternalInput").ap()
    bet_d = nc.dram_tensor("bet", [E], F32, kind="ExternalInput").ap()
    out_d = nc.dram_tensor("out", [T, E], F32, kind="ExternalOutput").ap()

    def bcast_row(dram_ap, n):
        return bass.AP(tensor=dram_ap.tensor, offset=0, ap=[[0, n]] + dram_ap.ap)

    with tile.TileContext(nc) as tc:
        with tc.tile_pool(name="persist", bufs=1) as P:
            # -------- bulk loads, in exact consumption order -----------------
            keyT = P.tile([128, ET, SEQ], BF16)
            wkT = P.tile([128, ET, KVE], BF16)
            queryT = P.tile([128, ET, T], BF16)
            wqT = P.tile([128, ET, E], BF16)
            valueT = P.tile([128, ET, SEQ], BF16)
            wvT = P.tile([128, ET, KVE], BF16)
            bq_s = P.tile([128, 8], F32)
            bk_s = P.tile([128, 2], F32)
            bvB = P.tile([128, KVE], F32)
            gamB = P.tile([128, E], F32)
            betB = P.tile([128, E], F32)
            # K-proj inputs in j-chunk order so the first matmuls fire early
            for e in range(ET):
                nc.sync.dma_start(out=wkT[:, e, :], in_=wkT_d[128 * e:128 * (e + 1), :])
            for j in range(4):
                for e in range(ET):
                    nc.sync.dma_start(out=keyT[:, e, 512 * j:512 * (j + 1)],
                                      in_=kT_d[128 * e:128 * (e + 1), 512 * j:512 * (j + 1)])
            for e in range(ET):
                nc.sync.dma_start(out=queryT[:, e, :], in_=qT_d[128 * e:128 * (e + 1), :])
                nc.sync.dma_start(out=wqT[:, e, 0:256], in_=wqT_d[128 * e:128 * (e + 1), 0:256])
            nc.sync.dma_start(out=bq_s, in_=bq_d)
            nc.sync.dma_start(out=bk_s, in_=bk_d)
            nc.sync.dma_start(out=bvB, in_=bcast_row(bv_d, 128))
            for e in range(ET):
                nc.sync.dma_start(out=wvT[:, e, :], in_=wvT_d[128 * e:128 * (e + 1), :])
            # valueT in s-tile-pair chunks, matching v-proj filler order
            for st2 in range(ST // 2):
                for e in range(ET):
                    nc.sync.dma_start(out=valueT[:, e, 256 * st2:256 * (st2 + 1)],
                                      in_=vT_d[128 * e:128 * (e + 1), 256 * st2:256 * (st2 + 1)])
            for e in range(ET):
                nc.sync.dma_start(out=wqT[:, e, 256:1024],
                                  in_=wqT_d[128 * e:128 * (e + 1), 256:1024])
            nc.sync.dma_start(out=gamB, in_=bcast_row(gam_d, 128))
            nc.sync.dma_start(out=betB, in_=bcast_row(bet_d, 128))

            ones_bf = P.tile([128, 1], BF16)
            nc.vector.memset(ones_bf, 1.0)
            eps_t = P.tile([128, 1], F32)
            nc.vector.memset(eps_t, EPS)

            q_sb = P.tile([128, 8, T], BF16)
            K_sb = P.tile([128, 2, SEQ], BF16)
            V_sb = P.tile([128, ST, KVH, HD], BF16)
            OUT = P.tile([128, 2, E], F32)

            with tc.tile_pool(name="psP", bufs=1, space="PSUM") as psP, \
                 tc.tile_pool(name="srA", bufs=1, space="PSUM") as srA, \
                 tc.tile_pool(name="srB", bufs=1, space="PSUM") as srB, \
                 tc.tile_pool(name="op", bufs=1, space="PSUM") as opp, \
                 tc.tile_pool(name="rp", bufs=1, space="PSUM") as rpp, \
                 tc.tile_pool(name="ering", bufs=4) as erp, \
                 tc.tile_pool(name="tail", bufs=2) as tlp:

                # ---- projection emitters (filler work units) ----
                def kproj(c, j, pk=None):
                    if pk is None:
                        pk = psP.tile([128, 512], F32, tag="pp", name=f"pk{c}{j}")
                    for e in range(ET):
                        nc.tensor.matmul(
                            pk, wkT[:, e, 128 * c:128 * (c + 1)],
                            keyT[:, e, 512 * j:512 * (j + 1)],
                            start=(e == 0), stop=(e == ET - 1))
                    nc.vector.tensor_scalar_add(
                        out=K_sb[:, c, 512 * j:512 * (j + 1)], in0=pk,
                        scalar1=bk_s[:, c:c + 1])

                def qproj(t, pq=None):
                    if pq is None:
                        pq = psP.tile([128, 512], F32, tag="pp", name=f"pq{t}")
                    for e in range(ET):
                        nc.tensor.matmul(
                            pq[:, 0:T], wqT[:, e, 128 * t:128 * (t + 1)],
                            queryT[:, e, :], start=(e == 0), stop=(e == ET - 1))
                    nc.vector.tensor_scalar_add(
                        out=q_sb[:, t, :], in0=pq[:, 0:T], scalar1=bq_s[:, t:t + 1])

                def vproj(st):
                    pv = psP.tile([128, 512], F32, tag="pp")
                    for e in range(ET):
                        nc.tensor.matmul(
                            pv[:, 0:KVE], valueT[:, e, 128 * st:128 * (st + 1)],
                            wvT[:, e, :], start=(e == 0), stop=(e == ET - 1))
                    nc.vector.tensor_add(
                        out=V_sb[:, st, :, :],
                        in0=pv[:, 0:KVE].rearrange("p (h d) -> p h d", h=KVH),
                        in1=bvB.rearrange("p (h d) -> p h d", h=KVH))

                # pre-stream projections: K channel-tile 0 and q tiles 0, 1,
                # rotated across the not-yet-used attention psum banks so the
                # PE never stalls on a single-buffer evacuation
                srA_pre = srA.tile([128, 3, 512], F32, tag="sA", name="srA_pre")
                srB_pre = srB.tile([128, 2, 512], F32, tag="sB", name="srB_pre")
                kproj(0, 0, srA_pre[:, 0, :])
                kproj(0, 1, srA_pre[:, 1, :])
                kproj(0, 2, srA_pre[:, 2, :])
                kproj(0, 3, srB_pre[:, 0, :])
                qproj(0, srB_pre[:, 1, :])
                qproj(1)

                # filler queue: v-proj is front-loaded (O of quad 0 needs it),
                # then K tile 1 (quads 2-3) and remaining q tiles (quads 1-3)
                fillers = [("v", st) for st in range(ST)] + \
                          [("q", t) for t in range(2, 8)] + \
                          [("k", j) for j in range(4)]
                f_idx = 0

                def run_filler():
                    nonlocal f_idx
                    kind, arg = fillers[f_idx]
                    f_idx += 1
                    if kind == "v":
                        vproj(arg)
                    elif kind == "k":
                        kproj(1, arg)
                    else:
                        qproj(arg)

                # ---- global attention slot stream ----
                # slot s = (p, st, hi): p = s // 32, st = (s % 32) // 2, hi = s % 2
                n_slots = 128
                groups = []
                i, size_a = 0, True
                while i < n_slots:
                    k = 3 if size_a else 2
                    groups.append(list(range(i, min(i + k, n_slots))))
                    i += k
                    size_a = not size_a

                def slot_info(s):
                    p, r = divmod(s, 32)
                    st, hi = divmod(r, 2)
                    return p, st, hi

                o_banks = {}
                r_banks = {}

                def emit_s(grp, sp, ep_):
                    for i_, s in enumerate(grp):
                        p, st, hi = slot_info(s)
                        if not hi:
                            nc.tensor.matmul(
                                sp[:, i_, :],
                                K_sb[0:64, KV_LO[p] // 2, 128 * st:128 * (st + 1)],
                                q_sb[0:64, 2 * p:2 * p + 2, :],
                                start=True, stop=True, tile_position=(0, 0))
                        else:
                            nc.tensor.matmul(
                                sp[:, i_, :],
                                K_sb[64:128, KV_HI[p] // 2, 128 * st:128 * (st + 1)],
                                q_sb[64:128, 2 * p:2 * p + 2, :],
                                start=True, stop=True, tile_position=(64, 0))

                def emit_or(grp, ep_):
                    tails = []
                    for i_, s in enumerate(grp):
                        p, st, hi = slot_info(s)
                        if p not in o_banks:
                            o_banks[p] = opp.tile([128, 512], F32, tag="o", name=f"o_ps{p}")
                            r_banks[p] = rpp.tile([128, 512], F32, tag="r", name=f"r_ps{p}")
                        kv = KV_HI[p] if hi else KV_LO[p]
                        cpos = 64 if hi else 0
                        nc.tensor.matmul(
                            o_banks[p][cpos:cpos + 64, :], V_sb[:, st, kv, :],
                            ep_[:, i_, :], start=(st == 0), stop=(st == ST - 1),
                            tile_position=(0, cpos), skip_group_check=True)
                        if st == ST - 1 and hi:
                            tails.append(p)
                    for i_, s in enumerate(grp):
                        p, st, hi = slot_info(s)
                        cpos = 64 if hi else 0
                        nc.tensor.matmul(
                            r_banks[p][cpos:cpos + 1, :], ones_bf,
                            ep_[:, i_, :], start=(st == 0), stop=(st == ST - 1),
                            tile_position=(0, cpos), skip_group_check=True)
                    for p in tails:
                        quad_tail(p)

                def quad_tail(p):
                    o_ps, r_ps = o_banks.pop(p), r_banks.pop(p)
                    r_st = tlp.tile([128, 512], BF16, tag="rst")
                    nc.vector.memset(r_st, 0.0)
                    nc.vector.tensor_copy(out=r_st[0:1, :], in_=r_ps[0:1, :])
                    nc.vector.tensor_copy(out=r_st[64:65, :], in_=r_ps[64:65, :])
                    o_st = tlp.tile([128, 512], BF16, tag="ost")
                    for k in range(4):
                        nc.vector.tensor_copy(out=o_st[:, 128 * k:128 * (k + 1)],
                                              in_=o_ps[:, 128 * k:128 * (k + 1)])
                    for k in range(4):
                        tt, ch = k % 2, k // 2
                        rt = tlp.tile([128, 128], BF16, tag="rt")
                        ot = tlp.tile([128, 128], BF16, tag="ot")
                        nc.scalar.dma_start(out=rt, in_=r_st[:, 128 * k:128 * (k + 1)],
                                            transpose=True)
                        nc.sync.dma_start(out=ot, in_=o_st[:, 128 * k:128 * (k + 1)],
                                          transpose=True)
                        rec = tlp.tile([128, 2], F32, tag="rec")
                        nc.vector.reciprocal(out=rec[:, 0:1], in_=rt[:, 0:1])
                        nc.vector.reciprocal(out=rec[:, 1:2], in_=rt[:, 64:65])
                        hE, hO = EH[2 * p + ch], OH[2 * p + ch]
                        nc.vector.tensor_scalar_mul(
                            out=OUT[:, tt, HD * hE:HD * (hE + 1)],
                            in0=ot[:, 0:64], scalar1=rec[:, 0:1])
                        nc.vector.tensor_scalar_mul(
                            out=OUT[:, tt, HD * hO:HD * (hO + 1)],
                            in0=ot[:, 64:128], scalar1=rec[:, 1:2])

                prev = None  # (grp, ep) awaiting O/R emission
                for gi, grp in enumerate(groups):
                    k = len(grp)
                    sp = (srA if k == 3 else srB).tile(
                        [128, k, 512], F32, tag="sA" if k == 3 else "sB")
                    ep_ = erp.tile([128, 3, 512], BF16, tag="e")
                    emit_s(grp, sp, ep_)
                    # filler cadence: v-proj st must land before O(quad0, st);
                    # front-load ~1.5 filler sets per group until exhausted
                    want = min(len(fillers), (3 * (gi + 1)) // 2)
                    while f_idx < want:
                        run_filler()
                    nc.scalar.activation(out=ep_[:, 0:k, :], in_=sp[:, 0:k, :],
                                         func=AF.Exp, scale=SCALE)
                    if prev is not None:
                        pgrp, pep = prev
                        emit_or(pgrp, pep)
                    prev = (grp, ep_)
                assert f_idx == len(fillers), f_idx
                # drain the final group
                pgrp, pep = prev
                emit_or(pgrp, pep)

            # ---------------- layernorm + store ----------------
            with tc.tile_pool(name="ln", bufs=2) as lnp:
                for tt in range(2):
                    stats = lnp.tile([128, 2, 6], F32, tag="stats")
                    nc.vector.bn_stats(out=stats[:, 0, :], in_=OUT[:, tt, 0:512])
                    nc.vector.bn_stats(out=stats[:, 1, :], in_=OUT[:, tt, 512:1024])
                    mv = lnp.tile([128, 2], F32, tag="mv")
                    nc.vector.bn_aggr(out=mv, in_=stats)
                    rstd = lnp.tile([128, 1], F32, tag="rstd")
                    nc.scalar.activation(out=rstd, in_=mv[:, 1:2], func=AF.Sqrt,
                                         bias=eps_t, scale=1.0)
                    nc.vector.reciprocal(out=rstd, in_=rstd)
                    y = lnp.tile([128, E], F32, tag="y")
                    nc.vector.tensor_scalar(out=y, in0=OUT[:, tt, :],
                                            scalar1=mv[:, 0:1], scalar2=rstd,
                                            op0=ALU.subtract, op1=ALU.mult)
                    z = lnp.tile([128, E], F32, tag="z")
                    nc.vector.tensor_mul(out=z, in0=y, in1=gamB)
                    nc.vector.tensor_add(out=z, in0=z, in1=betB)
                    nc.sync.dma_start(out=out_d[128 * tt:128 * (tt + 1), :], in_=z)

    nc.compile()
    return nc


def _prep_inputs(query, key, value, Wq, bq, Wk, bk, Wv, bv, gamma, beta):
    bf = ml_dtypes.bfloat16
    query, key, value = np.asarray(query), np.asarray(key), np.asarray(value)
    Wq, Wk, Wv = np.asarray(Wq), np.asarray(Wk), np.asarray(Wv)
    bq, bk, bv = np.asarray(bq), np.asarray(bk), np.asarray(bv)
    qT = np.ascontiguousarray(query[0].T.astype(bf))
    kT = np.ascontiguousarray(key[0].T.astype(bf))
    vT = np.ascontiguousarray(value[0].T.astype(bf))
    wqT = np.ascontiguousarray(Wq.T[:, COLPERM].astype(bf))
    wkT = np.ascontiguousarray(Wk.T.astype(bf))
    wvT = np.ascontiguousarray(Wv.T.astype(bf))
    bq_p = np.ascontiguousarray(bq[COLPERM].reshape(8, 128).T.astype(np.float32))
    bk_p = np.ascontiguousarray(bk.reshape(2, 128).T.astype(np.float32))
    common = {
        "kT": kT, "vT": vT, "wqT": wqT, "wkT": wkT, "wvT": wvT,
        "bq": bq_p, "bk": bk_p, "bv": np.asarray(bv, np.float32),
        "gam": np.asarray(gamma, np.float32), "bet": np.asarray(beta, np.float32),
    }
    in_maps = []
    for c in range(N_CORES):
        m = dict(common)
        m["qT"] = np.ascontiguousarray(qT[:, T * c:T * (c + 1)])
        in_maps.append(m)
    return in_maps


def run(inputs, trace=False):
    if "nc" not in _CACHE:
        _CACHE["nc"] = _build()
    nc = _CACHE["nc"]
    in_maps = _prep_inputs(**inputs)
    res = bass_utils.run_bass_kernel_spmd(
        nc, in_maps, core_ids=list(range(N_CORES)), trace=trace)
    out = np.empty((1, SEQ, E), np.float32)
    for c in range(N_CORES):
        out[0, T * c:T * (c + 1), :] = res.results[c]["out"]
    return out, res


def kernel(**inputs):
    out, _ = run(inputs, trace=False)
    return out


# revision 13
# speedup vs baseline: 1.0107x; 1.0107x over previous
"""MultiHead GQA (16 q heads / 4 kv heads, E=1024, n=2048, b=1) on 8 TRN2 cores.

Strategy: shard the 2048 query positions across the 8 cores (256 each); every
core computes the full K/V projections for all 2048 key positions (replicating
that small matmul is far cheaper than any collective at this size), runs
attention + layernorm for its own 256 tokens, and writes its 256x1024 slice.

All heavy matmuls run in bf16 (fp32 PSUM accumulation).  The host pre-
transposes and pre-casts the operands so that every matmul contraction
dimension lands on SBUF partitions with no on-device transposes; the only
on-device transposes are tiny bf16 DMA X-bar transposes of the attention
output (64x256 per head) and of the softmax denominators.

The emission order software-pipelines PE: the attention stream is a single
sequence of (pair, s-tile) slot groups; each group's S matmuls are followed
by projection "filler" matmuls and the *previous* group's O/R matmuls, so
the (in-order) PE queue never blocks on the exp that gates O/R.

RoPE in the reference is the identity for b=1 (seq index = batch index = 0,
so cos=1 / sin=0 exactly); it is therefore omitted.
"""

import numpy as np
import ml_dtypes

import concourse.bass as bass
import concourse.bacc as bacc
import concourse.tile as tile
from concourse import mybir
from concourse import bass_utils

F32 = mybir.dt.float32
BF16 = mybir.dt.bfloat16
AF = mybir.ActivationFunctionType
ALU = mybir.AluOpType

N_CORES = 8
E = 1024
QH = 16
KVH = 4
HD = 64
KVE = KVH * HD
SEQ = 2048
T = SEQ // N_CORES   # 256 query tokens per core
ST = SEQ // 128      # 16 key s-tiles
ET = E // 128        # 8 contraction e-tiles
EPS = 1e-5
SCALE = 1.0 / (HD ** 0.5)

# Head bookkeeping: q-proj channel-tile t packs head EH[t] in partitions 0-63
# and head OH[t] in partitions 64-127.  EH heads use even kv heads (0, 2),
# OH heads use odd kv heads (1, 3), which matches the natural K-proj layout
# (K channel-tile 0 = kv0|kv1, tile 1 = kv2|kv3) with no partition shifts.
EH = [0, 1, 2, 3, 8, 9, 10, 11]
OH = [4, 5, 6, 7, 12, 13, 14, 15]
KV_LO = [EH[2 * p] // 4 for p in range(4)]   # [0, 0, 2, 2]
KV_HI = [OH[2 * p] // 4 for p in range(4)]   # [1, 1, 3, 3]

COLPERM = np.concatenate(
    [np.r_[EH[t] * HD:(EH[t] + 1) * HD, OH[t] * HD:(OH[t] + 1) * HD]
     for t in range(8)])

_CACHE = {}


def _build():
    nc = bacc.Bacc("TRN2", target_bir_lowering=False, debug=False)

    qT_d = nc.dram_tensor("qT", [E, T], BF16, kind="ExternalInput").ap()
    kT_d = nc.dram_tensor("kT", [E, SEQ], BF16, kind="ExternalInput").ap()
    vT_d = nc.dram_tensor("vT", [E, SEQ], BF16, kind="ExternalInput").ap()
    wqT_d = nc.dram_tensor("wqT", [E, E], BF16, kind="ExternalInput").ap()
    wkT_d = nc.dram_tensor("wkT", [E, KVE], BF16, kind="ExternalInput").ap()
    wvT_d = nc.dram_tensor("wvT", [E, KVE], BF16, kind="ExternalInput").ap()
    bq_d = nc.dram_tensor("bq", [128, 8], F32, kind="ExternalInput").ap()
    bk_d = nc.dram_tensor("bk", [128, 2], F32, kind="ExternalInput").ap()
    bv_d = nc.dram_tensor("bv", [KVE], F32, kind="ExternalInput").ap()
    gam_d = nc.dram_tensor("gam", [E], F32, kind="ExternalInput").ap()
    bet_d = nc.dram_tensor("bet", [E], F32, kind="ExternalInput").ap()
    out_d = nc.dram_tensor("out", [T, E], F32, kind="ExternalOutput").ap()

    def bcast_row(dram_ap, n):
        return bass.AP(tensor=dram_ap.tensor, offset=0, ap=[[0, n]] + dram_ap.ap)

    with tile.TileContext(nc) as tc:
        with tc.tile_pool(name="persist", bufs=1) as P:
            # -------- bulk loads, in exact consumption order -----------------
            keyT = P.tile([128, ET, SEQ], BF16)
            wkT = P.tile([128, ET, KVE], BF16)
            queryT = P.tile([128, ET, T], BF16)
            wqT = P.tile([128, ET, E], BF16)
            valueT = P.tile([128, ET, SEQ], BF16)
            wvT = P.tile([128, ET, KVE], BF16)
            bq_s = P.tile([128, 8], F32)
            bk_s = P.tile([128, 2], F32)
            bvB = P.tile([128, KVE], F32)
            gamB = P.tile([128, E], F32)
            betB = P.tile([128, E], F32)
            # Prestream prefix: wkT, keyT j0/j1, queryT, wqT tiles 0-1
            for e in range(ET):
                nc.sync.dma_start(out=wkT[:, e, :], in_=wkT_d[128 * e:128 * (e + 1), :])
            for j in range(2):
                for e in range(ET):
                    nc.sync.dma_start(out=keyT[:, e, 512 * j:512 * (j + 1)],
                                      in_=kT_d[128 * e:128 * (e + 1), 512 * j:512 * (j + 1)])
            for e in range(ET):
                nc.sync.dma_start(out=queryT[:, e, :], in_=qT_d[128 * e:128 * (e + 1), :])
                nc.sync.dma_start(out=wqT[:, e, 0:256], in_=wqT_d[128 * e:128 * (e + 1), 0:256])
            for j in range(2, 4):
                for e in range(ET):
                    nc.sync.dma_start(out=keyT[:, e, 512 * j:512 * (j + 1)],
                                      in_=kT_d[128 * e:128 * (e + 1), 512 * j:512 * (j + 1)])
            nc.sync.dma_start(out=bq_s, in_=bq_d)
            nc.sync.dma_start(out=bk_s, in_=bk_d)
            nc.sync.dma_start(out=bvB, in_=bcast_row(bv_d, 128))
            for e in range(ET):
                nc.sync.dma_start(out=wvT[:, e, :], in_=wvT_d[128 * e:128 * (e + 1), :])
            for e in range(ET):
                nc.sync.dma_start(out=valueT[:, e, :], in_=vT_d[128 * e:128 * (e + 1), :])
            for e in range(ET):
                nc.sync.dma_start(out=wqT[:, e, 256:1024],
                                  in_=wqT_d[128 * e:128 * (e + 1), 256:1024])
            nc.sync.dma_start(out=gamB, in_=bcast_row(gam_d, 128))
            nc.sync.dma_start(out=betB, in_=bcast_row(bet_d, 128))

            ones_bf = P.tile([128, 1], BF16)
            nc.vector.memset(ones_bf, 1.0)
            eps_t = P.tile([128, 1], F32)
            nc.vector.memset(eps_t, EPS)

            q_sb = P.tile([128, 8, T], BF16)
            K_sb = P.tile([128, 2, SEQ], BF16)
            V_sb = P.tile([128, ST, KVH, HD], BF16)
            OUT = P.tile([128, 2, E], F32)

            with tc.tile_pool(name="psP", bufs=1, space="PSUM") as psP, \
                 tc.tile_pool(name="srA", bufs=1, space="PSUM") as srA, \
                 tc.tile_pool(name="srB", bufs=1, space="PSUM") as srB, \
                 tc.tile_pool(name="op", bufs=1, space="PSUM") as opp, \
                 tc.tile_pool(name="rp", bufs=1, space="PSUM") as rpp, \
                 tc.tile_pool(name="ering", bufs=4) as erp, \
                 tc.tile_pool(name="tail", bufs=2) as tlp:

                # ---- projection emitters (filler work units) ----
                def kproj(c, j, pk=None):
                    if pk is None:
                        pk = psP.tile([128, 512], F32, tag="pp", name=f"pk{c}{j}")
                    for e in range(ET):
                        nc.tensor.matmul(
                            pk, wkT[:, e, 128 * c:128 * (c + 1)],
                            keyT[:, e, 512 * j:512 * (j + 1)],
                            start=(e == 0), stop=(e == ET - 1))
                    nc.vector.tensor_scalar_add(
                        out=K_sb[:, c, 512 * j:512 * (j + 1)], in0=pk,
                        scalar1=bk_s[:, c:c + 1])

                def qproj(t, pq=None):
                    if pq is None:
                        pq = psP.tile([128, 512], F32, tag="pp", name=f"pq{t}")
                    for e in range(ET):
                        nc.tensor.matmul(
                            pq[:, 0:T], wqT[:, e, 128 * t:128 * (t + 1)],
                            queryT[:, e, :], start=(e == 0), stop=(e == ET - 1))
                    nc.vector.tensor_scalar_add(
                        out=q_sb[:, t, :], in0=pq[:, 0:T], scalar1=bq_s[:, t:t + 1])

                def vproj(st):
                    pv = psP.tile([128, 512], F32, tag="pp")
                    for e in range(ET):
                        nc.tensor.matmul(
                            pv[:, 0:KVE], valueT[:, e, 128 * st:128 * (st + 1)],
                            wvT[:, e, :], start=(e == 0), stop=(e == ET - 1))
                    nc.vector.tensor_add(
                        out=V_sb[:, st, :, :],
                        in0=pv[:, 0:KVE].rearrange("p (h d) -> p h d", h=KVH),
                        in1=bvB.rearrange("p (h d) -> p h d", h=KVH))

                # pre-stream projections: K channel-tile 0 and q tiles 0, 1,
                # rotated across the not-yet-used attention psum banks so the
                # PE never stalls on a single-buffer evacuation
                srA_pre = srA.tile([128, 3, 512], F32, tag="sA", name="srA_pre")
                srB_pre = srB.tile([128, 2, 512], F32, tag="sB", name="srB_pre")
                kproj(0, 0, srA_pre[:, 0, :])
                kproj(0, 1, srA_pre[:, 1, :])
                qproj(0, srA_pre[:, 2, :])
                qproj(1, srB_pre[:, 0, :])
                kproj(0, 2, srB_pre[:, 1, :])

                # filler queue: v-proj is front-loaded (O of quad 0 needs it),
                # then K tile 1 (quads 2-3) and remaining q tiles (quads 1-3)
                fillers = [("k0", 3), ("q", 2), ("q", 3)] + \
                          [("v", st) for st in range(ST)] + \
                          [("q", t) for t in range(4, 8)] + \
                          [("k1", j) for j in range(4)]
                f_pos = {f: i for i, f in enumerate(fillers)}
                f_idx = 0

                def run_filler():
                    nonlocal f_idx
                    kind, arg = fillers[f_idx]
                    f_idx += 1
                    if kind == "v":
                        vproj(arg)
                    elif kind == "k1":
                        kproj(1, arg)
                    elif kind == "k0":
                        kproj(0, arg)
                    else:
                        qproj(arg)

                def ensure(*needs):
                    # emit fillers up to and including every needed one
                    idxs = [f_pos[n] for n in needs if n in f_pos]
                    while idxs and f_idx <= max(idxs):
                        run_filler()

                def s_needs(s):
                    p, st, hi = slot_info(s)
                    c = (KV_HI[p] if hi else KV_LO[p]) // 2
                    return [("q", 2 * p), ("q", 2 * p + 1),
                            ("k0", st // 4) if c == 0 else ("k1", st // 4)]

                # ---- global attention slot stream ----
                # slot s = (p, st, hi): p = s // 32, st = (s % 32) // 2, hi = s % 2
                n_slots = 128
                groups = []
                i, size_a = 0, True
                while i < n_slots:
                    k = 3 if size_a else 2
                    groups.append(list(range(i, min(i + k, n_slots))))
                    i += k
                    size_a = not size_a

                def slot_info(s):
                    p, r = divmod(s, 32)
                    st, hi = divmod(r, 2)
                    return p, st, hi

                o_banks = {}
                r_banks = {}

                def emit_s(grp, sp, ep_):
                    for i_, s in enumerate(grp):
                        p, st, hi = slot_info(s)
                        if not hi:
                            nc.tensor.matmul(
                                sp[:, i_, :],
                                K_sb[0:64, KV_LO[p] // 2, 128 * st:128 * (st + 1)],
                                q_sb[0:64, 2 * p:2 * p + 2, :],
                                start=True, stop=True, tile_position=(0, 0))
                        else:
                            nc.tensor.matmul(
                                sp[:, i_, :],
                                K_sb[64:128, KV_HI[p] // 2, 128 * st:128 * (st + 1)],
                                q_sb[64:128, 2 * p:2 * p + 2, :],
                                start=True, stop=True, tile_position=(64, 0))

                def emit_or(grp, ep_):
                    tails = []
                    for i_, s in enumerate(grp):
                        p, st, hi = slot_info(s)
                        if p not in o_banks:
                            o_banks[p] = opp.tile([128, 512], F32, tag="o", name=f"o_ps{p}")
                            r_banks[p] = rpp.tile([128, 512], F32, tag="r", name=f"r_ps{p}")
                        kv = KV_HI[p] if hi else KV_LO[p]
                        cpos = 64 if hi else 0
                        nc.tensor.matmul(
                            o_banks[p][cpos:cpos + 64, :], V_sb[:, st, kv, :],
                            ep_[:, i_, :], start=(st == 0), stop=(st == ST - 1),
                            tile_position=(0, cpos), skip_group_check=True)
                        if st == ST - 1 and hi:
                            tails.append(p)
                    for i_, s in enumerate(grp):
                        p, st, hi = slot_info(s)
                        cpos = 64 if hi else 0
                        nc.tensor.matmul(
                            r_banks[p][cpos:cpos + 1, :], ones_bf,
                            ep_[:, i_, :], start=(st == 0), stop=(st == ST - 1),
                            tile_position=(0, cpos), skip_group_check=True)
                    for p in tails:
                        quad_tail(p)

                def quad_tail(p):
                    o_ps, r_ps = o_banks.pop(p), r_banks.pop(p)
                    r_st = tlp.tile([128, 512], BF16, tag="rst")
                    nc.vector.memset(r_st, 0.0)
                    nc.vector.tensor_copy(out=r_st[0:1, :], in_=r_ps[0:1, :])
                    nc.vector.tensor_copy(out=r_st[64:65, :], in_=r_ps[64:65, :])
                    o_st = tlp.tile([128, 512], BF16, tag="ost")
                    for k in range(4):
                        nc.vector.tensor_copy(out=o_st[:, 128 * k:128 * (k + 1)],
                                              in_=o_ps[:, 128 * k:128 * (k + 1)])
                    for k in range(4):
                        tt, ch = k % 2, k // 2
                        rt = tlp.tile([128, 128], BF16, tag="rt")
                        ot = tlp.tile([128, 128], BF16, tag="ot")
                        nc.scalar.dma_start(out=rt, in_=r_st[:, 128 * k:128 * (k + 1)],
                                            transpose=True)
                        nc.sync.dma_start(out=ot, in_=o_st[:, 128 * k:128 * (k + 1)],
                                          transpose=True)
                        rec = tlp.tile([128, 2], F32, tag="rec")
                        nc.vector.reciprocal(out=rec[:, 0:1], in_=rt[:, 0:1])
                        nc.vector.reciprocal(out=rec[:, 1:2], in_=rt[:, 64:65])
                        hE, hO = EH[2 * p + ch], OH[2 * p + ch]
                        nc.vector.tensor_scalar_mul(
                            out=OUT[:, tt, HD * hE:HD * (hE + 1)],
                            in0=ot[:, 0:64], scalar1=rec[:, 0:1])
                        nc.vector.tensor_scalar_mul(
                            out=OUT[:, tt, HD * hO:HD * (hO + 1)],
                            in0=ot[:, 64:128], scalar1=rec[:, 1:2])

                prev = None  # (grp, ep) awaiting O/R emission
                for gi, grp in enumerate(groups):
                    for s in grp:
                        ensure(*s_needs(s))
                    k = len(grp)
                    sp = (srA if k == 3 else srB).tile(
                        [128, k, 512], F32, tag="sA" if k == 3 else "sB")
                    ep_ = erp.tile([128, 3, 512], BF16, tag="e")
                    emit_s(grp, sp, ep_)
                    # keep draining fillers at ~1.3 sets per group
                    want = min(len(fillers), (4 * (gi + 1)) // 3)
                    while f_idx < want:
                        run_filler()
                    nc.scalar.activation(out=ep_[:, 0:k, :], in_=sp[:, 0:k, :],
                                         func=AF.Exp, scale=SCALE)
                    if prev is not None:
                        pgrp, pep = prev
                        ensure(*[("v", slot_info(s)[1]) for s in pgrp])
                        emit_or(pgrp, pep)
                    prev = (grp, ep_)
                # drain the final group
                pgrp, pep = prev
                ensure(*[("v", slot_info(s)[1]) for s in pgrp])
                emit_or(pgrp, pep)
                while f_idx < len(fillers):
                    run_filler()

            # ---------------- layernorm + store ----------------
            with tc.tile_pool(name="ln", bufs=2) as lnp:
                for tt in range(2):
                    stats = lnp.tile([128, 2, 6], F32, tag="stats")
                    nc.vector.bn_stats(out=stats[:, 0, :], in_=OUT[:, tt, 0:512])
                    nc.vector.bn_stats(out=stats[:, 1, :], in_=OUT[:, tt, 512:1024])
                    mv = lnp.tile([128, 2], F32, tag="mv")
                    nc.vector.bn_aggr(out=mv, in_=stats)
                    rstd = lnp.tile([128, 1], F32, tag="rstd")
                    nc.scalar.activation(out=rstd, in_=mv[:, 1:2], func=AF.Sqrt,
                                         bias=eps_t, scale=1.0)
                    nc.vector.reciprocal(out=rstd, in_=rstd)
                    y = lnp.tile([128, E], F32, tag="y")
                    nc.vector.tensor_scalar(out=y, in0=OUT[:, tt, :],
                                            scalar1=mv[:, 0:1], scalar2=rstd,
                                            op0=ALU.subtract, op1=ALU.mult)
                    z = lnp.tile([128, E], F32, tag="z")
                    nc.vector.tensor_mul(out=z, in0=y, in1=gamB)
                    nc.vector.tensor_add(out=z, in0=z, in1=betB)
                    nc.sync.dma_start(out=out_d[128 * tt:128 * (tt + 1), :], in_=z)

    nc.compile()
    return nc


def _prep_inputs(query, key, value, Wq, bq, Wk, bk, Wv, bv, gamma, beta):
    bf = ml_dtypes.bfloat16
    query, key, value = np.asarray(query), np.asarray(key), np.asarray(value)
    Wq, Wk, Wv = np.asarray(Wq), np.asarray(Wk), np.asarray(Wv)
    bq, bk, bv = np.asarray(bq), np.asarray(bk), np.asarray(bv)
    qT = np.ascontiguousarray(query[0].T.astype(bf))
    kT = np.ascontiguousarray(key[0].T.astype(bf))
    vT = np.ascontiguousarray(value[0].T.astype(bf))
    wqT = np.ascontiguousarray(Wq.T[:, COLPERM].astype(bf))
    wkT = np.ascontiguousarray(Wk.T.astype(bf))
    wvT = np.ascontiguousarray(Wv.T.astype(bf))
    bq_p = np.ascontiguousarray(bq[COLPERM].reshape(8, 128).T.astype(np.float32))
    bk_p = np.ascontiguousarray(bk.reshape(2, 128).T.astype(np.float32))
    common = {
        "kT": kT, "vT": vT, "wqT": wqT, "wkT": wkT, "wvT": wvT,
        "bq": bq_p, "bk": bk_p, "bv": np.asarray(bv, np.float32),
        "gam": np.asarray(gamma, np.float32), "bet": np.asarray(beta, np.float32),
    }
    in_maps = []
    for c in range(N_CORES):
        m = dict(common)
        m["qT"] = np.ascontiguousarray(qT[:, T * c:T * (c + 1)])
        in_maps.append(m)
    return in_maps


def run(inputs, trace=False):
    if "nc" not in _CACHE:
        _CACHE["nc"] = _build()
    nc = _CACHE["nc"]
    in_maps = _prep_inputs(**inputs)
    res = bass_utils.run_bass_kernel_spmd(
        nc, in_maps, core_ids=list(range(N_CORES)), trace=trace)
    out = np.empty((1, SEQ, E), np.float32)
    for c in range(N_CORES):
        out[0, T * c:T * (c + 1), :] = res.results[c]["out"]
    return out, res


def kernel(**inputs):
    out, _ = run(inputs, trace=False)
    return out


# revision 14
# speedup vs baseline: 1.1222x; 1.1103x over previous
"""MultiHead GQA (16 q heads / 4 kv heads, E=1024, n=2048, b=1) on 8 TRN2 cores.

Strategy: shard the 2048 query positions across the 8 cores (256 each); every
core computes the full K/V projections for all 2048 key positions (replicating
that small matmul is far cheaper than any collective at this size), runs
attention + layernorm for its own 256 tokens, and writes its 256x1024 slice.

All heavy matmuls run in bf16 (fp32 PSUM accumulation).  The host pre-
transposes and pre-casts the operands so that every matmul contraction
dimension lands on SBUF partitions with no on-device transposes; the only
on-device transposes are tiny bf16 DMA X-bar transposes of the attention
output (64x256 per head) and of the softmax denominators.

The emission order software-pipelines PE: the attention stream is a single
sequence of (pair, s-tile) slot groups; each group's S matmuls are followed
by projection "filler" matmuls and the *previous* group's O/R matmuls, so
the (in-order) PE queue never blocks on the exp that gates O/R.

RoPE in the reference is the identity for b=1 (seq index = batch index = 0,
so cos=1 / sin=0 exactly); it is therefore omitted.
"""

import numpy as np
import ml_dtypes

import concourse.bass as bass
import concourse.bacc as bacc
import concourse.tile as tile
from concourse import mybir
from concourse import bass_utils

F32 = mybir.dt.float32
BF16 = mybir.dt.bfloat16
AF = mybir.ActivationFunctionType
ALU = mybir.AluOpType

N_CORES = 8
E = 1024
QH = 16
KVH = 4
HD = 64
KVE = KVH * HD
SEQ = 2048
T = SEQ // N_CORES   # 256 query tokens per core
ST = SEQ // 128      # 16 key s-tiles
ET = E // 128        # 8 contraction e-tiles
EPS = 1e-5
SCALE = 1.0 / (HD ** 0.5)

# Head bookkeeping: q-proj channel-tile t packs head EH[t] in partitions 0-63
# and head OH[t] in partitions 64-127.  EH heads use even kv heads (0, 2),
# OH heads use odd kv heads (1, 3), which matches the natural K-proj layout
# (K channel-tile 0 = kv0|kv1, tile 1 = kv2|kv3) with no partition shifts.
EH = [0, 1, 2, 3, 8, 9, 10, 11]
OH = [4, 5, 6, 7, 12, 13, 14, 15]
KV_LO = [EH[2 * p] // 4 for p in range(4)]   # [0, 0, 2, 2]
KV_HI = [OH[2 * p] // 4 for p in range(4)]   # [1, 1, 3, 3]

COLPERM = np.concatenate(
    [np.r_[EH[t] * HD:(EH[t] + 1) * HD, OH[t] * HD:(OH[t] + 1) * HD]
     for t in range(8)])

_CACHE = {}


def _build():
    nc = bacc.Bacc("TRN2", target_bir_lowering=False, debug=False)

    qT_d = nc.dram_tensor("qT", [E, T], BF16, kind="ExternalInput").ap()
    kT_d = nc.dram_tensor("kT", [E, SEQ], BF16, kind="ExternalInput").ap()
    vT_d = nc.dram_tensor("vT", [E, SEQ], BF16, kind="ExternalInput").ap()
    wqT_d = nc.dram_tensor("wqT", [E, E], BF16, kind="ExternalInput").ap()
    wkT_d = nc.dram_tensor("wkT", [E, KVE], BF16, kind="ExternalInput").ap()
    wvT_d = nc.dram_tensor("wvT", [E, KVE], BF16, kind="ExternalInput").ap()
    bq_d = nc.dram_tensor("bq", [128, 8], F32, kind="ExternalInput").ap()
    bk_d = nc.dram_tensor("bk", [128, 2], F32, kind="ExternalInput").ap()
    bv_d = nc.dram_tensor("bv", [KVE], F32, kind="ExternalInput").ap()
    gam_d = nc.dram_tensor("gam", [E], F32, kind="ExternalInput").ap()
    bet_d = nc.dram_tensor("bet", [E], F32, kind="ExternalInput").ap()
    out_d = nc.dram_tensor("out", [T, E], F32, kind="ExternalOutput").ap()

    def bcast_row(dram_ap, n):
        return bass.AP(tensor=dram_ap.tensor, offset=0, ap=[[0, n]] + dram_ap.ap)

    with tile.TileContext(nc) as tc:
        with tc.tile_pool(name="persist", bufs=1) as P:
            # -------- bulk loads, in exact consumption order -----------------
            keyT = P.tile([128, ET, SEQ], BF16)
            wkT = P.tile([128, ET, KVE], BF16)
            queryT = P.tile([128, ET, T], BF16)
            wqT = P.tile([128, ET, E], BF16)
            valueT = P.tile([128, ET, SEQ], BF16)
            wvT = P.tile([128, ET, KVE], BF16)
            bq_s = P.tile([128, 8], F32)
            bk_s = P.tile([128, 2], F32)
            bvB = P.tile([128, KVE], F32)
            gamB = P.tile([128, E], F32)
            betB = P.tile([128, E], F32)
            # Consolidated strided loads ([p t s] views of the row-major
            # DRAM tensors) -- few big DMAs, issued in consumption order.
            kT_v = kT_d.rearrange("(t p) s -> p t s", p=128)
            qT_v = qT_d.rearrange("(t p) s -> p t s", p=128)
            vT_v = vT_d.rearrange("(t p) s -> p t s", p=128)
            wqT_v = wqT_d.rearrange("(t p) s -> p t s", p=128)
            wkT_v = wkT_d.rearrange("(t p) s -> p t s", p=128)
            wvT_v = wvT_d.rearrange("(t p) s -> p t s", p=128)
            nc.sync.dma_start(out=wkT, in_=wkT_v)
            nc.sync.dma_start(out=keyT[:, :, 0:1024], in_=kT_v[:, :, 0:1024])
            nc.sync.dma_start(out=queryT, in_=qT_v)
            nc.sync.dma_start(out=wqT[:, :, 0:256], in_=wqT_v[:, :, 0:256])
            nc.sync.dma_start(out=keyT[:, :, 1024:2048], in_=kT_v[:, :, 1024:2048])
            nc.sync.dma_start(out=bq_s, in_=bq_d)
            nc.sync.dma_start(out=bk_s, in_=bk_d)
            nc.sync.dma_start(out=bvB, in_=bcast_row(bv_d, 128))
            nc.sync.dma_start(out=wvT, in_=wvT_v)
            nc.sync.dma_start(out=valueT[:, :, 0:1024], in_=vT_v[:, :, 0:1024])
            nc.sync.dma_start(out=valueT[:, :, 1024:2048], in_=vT_v[:, :, 1024:2048])
            nc.sync.dma_start(out=wqT[:, :, 256:1024], in_=wqT_v[:, :, 256:1024])
            nc.sync.dma_start(out=gamB, in_=bcast_row(gam_d, 128))
            nc.sync.dma_start(out=betB, in_=bcast_row(bet_d, 128))

            ones_bf = P.tile([128, 1], BF16)
            nc.vector.memset(ones_bf, 1.0)
            eps_t = P.tile([128, 1], F32)
            nc.vector.memset(eps_t, EPS)

            q_sb = P.tile([128, 8, T], BF16)
            K_sb = P.tile([128, 2, SEQ], BF16)
            V_sb = P.tile([128, ST, KVH, HD], BF16)
            OUT = P.tile([128, 2, E], F32)

            with tc.tile_pool(name="psP", bufs=1, space="PSUM") as psP, \
                 tc.tile_pool(name="srA", bufs=1, space="PSUM") as srA, \
                 tc.tile_pool(name="srB", bufs=1, space="PSUM") as srB, \
                 tc.tile_pool(name="op", bufs=1, space="PSUM") as opp, \
                 tc.tile_pool(name="rp", bufs=1, space="PSUM") as rpp, \
                 tc.tile_pool(name="ering", bufs=4) as erp, \
                 tc.tile_pool(name="tail", bufs=2) as tlp:

                # ---- projection emitters (filler work units) ----
                def kproj(c, j, pk=None):
                    if pk is None:
                        pk = psP.tile([128, 512], F32, tag="pp", name=f"pk{c}{j}")
                    for e in range(ET):
                        nc.tensor.matmul(
                            pk, wkT[:, e, 128 * c:128 * (c + 1)],
                            keyT[:, e, 512 * j:512 * (j + 1)],
                            start=(e == 0), stop=(e == ET - 1))
                    nc.vector.tensor_scalar_add(
                        out=K_sb[:, c, 512 * j:512 * (j + 1)], in0=pk,
                        scalar1=bk_s[:, c:c + 1])

                def qproj(t, pq=None):
                    if pq is None:
                        pq = psP.tile([128, 512], F32, tag="pp", name=f"pq{t}")
                    for e in range(ET):
                        nc.tensor.matmul(
                            pq[:, 0:T], wqT[:, e, 128 * t:128 * (t + 1)],
                            queryT[:, e, :], start=(e == 0), stop=(e == ET - 1))
                    nc.vector.tensor_scalar_add(
                        out=q_sb[:, t, :], in0=pq[:, 0:T], scalar1=bq_s[:, t:t + 1])

                def vproj(st):
                    pv = psP.tile([128, 512], F32, tag="pp")
                    for e in range(ET):
                        nc.tensor.matmul(
                            pv[:, 0:KVE], valueT[:, e, 128 * st:128 * (st + 1)],
                            wvT[:, e, :], start=(e == 0), stop=(e == ET - 1))
                    nc.vector.tensor_add(
                        out=V_sb[:, st, :, :],
                        in0=pv[:, 0:KVE].rearrange("p (h d) -> p h d", h=KVH),
                        in1=bvB.rearrange("p (h d) -> p h d", h=KVH))

                # pre-stream projections: K channel-tile 0 and q tiles 0, 1,
                # rotated across the not-yet-used attention psum banks so the
                # PE never stalls on a single-buffer evacuation
                srA_pre = srA.tile([128, 3, 512], F32, tag="sA", name="srA_pre")
                srB_pre = srB.tile([128, 2, 512], F32, tag="sB", name="srB_pre")
                kproj(0, 0, srA_pre[:, 0, :])
                kproj(0, 1, srA_pre[:, 1, :])
                qproj(0, srA_pre[:, 2, :])
                qproj(1, srB_pre[:, 0, :])
                kproj(0, 2, srB_pre[:, 1, :])

                # filler queue: v-proj is front-loaded (O of quad 0 needs it),
                # then K tile 1 (quads 2-3) and remaining q tiles (quads 1-3)
                fillers = [("k0", 3), ("q", 2), ("q", 3)] + \
                          [("v", st) for st in range(ST)] + \
                          [("q", t) for t in range(4, 8)] + \
                          [("k1", j) for j in range(4)]
                f_pos = {f: i for i, f in enumerate(fillers)}
                f_idx = 0

                def run_filler():
                    nonlocal f_idx
                    kind, arg = fillers[f_idx]
                    f_idx += 1
                    if kind == "v":
                        vproj(arg)
                    elif kind == "k1":
                        kproj(1, arg)
                    elif kind == "k0":
                        kproj(0, arg)
                    else:
                        qproj(arg)

                def ensure(*needs):
                    # emit fillers up to and including every needed one
                    idxs = [f_pos[n] for n in needs if n in f_pos]
                    while idxs and f_idx <= max(idxs):
                        run_filler()

                def s_needs(s):
                    p, st, hi = slot_info(s)
                    c = (KV_HI[p] if hi else KV_LO[p]) // 2
                    return [("q", 2 * p), ("q", 2 * p + 1),
                            ("k0", st // 4) if c == 0 else ("k1", st // 4)]

                # ---- global attention slot stream ----
                # slot s = (p, st, hi): p = s // 32, st = (s % 32) // 2, hi = s % 2
                n_slots = 128
                groups = []
                i, size_a = 0, True
                while i < n_slots:
                    k = 3 if size_a else 2
                    groups.append(list(range(i, min(i + k, n_slots))))
                    i += k
                    size_a = not size_a

                def slot_info(s):
                    p, r = divmod(s, 32)
                    st, hi = divmod(r, 2)
                    return p, st, hi

                o_banks = {}
                r_banks = {}

                def emit_s(grp, sp, ep_):
                    for i_, s in enumerate(grp):
                        p, st, hi = slot_info(s)
                        if not hi:
                            nc.tensor.matmul(
                                sp[:, i_, :],
                                K_sb[0:64, KV_LO[p] // 2, 128 * st:128 * (st + 1)],
                                q_sb[0:64, 2 * p:2 * p + 2, :],
                                start=True, stop=True, tile_position=(0, 0))
                        else:
                            nc.tensor.matmul(
                                sp[:, i_, :],
                                K_sb[64:128, KV_HI[p] // 2, 128 * st:128 * (st + 1)],
                                q_sb[64:128, 2 * p:2 * p + 2, :],
                                start=True, stop=True, tile_position=(64, 0))

                def emit_or(grp, ep_):
                    tails = []
                    for i_, s in enumerate(grp):
                        p, st, hi = slot_info(s)
                        if p not in o_banks:
                            o_banks[p] = opp.tile([128, 512], F32, tag="o", name=f"o_ps{p}")
                            r_banks[p] = rpp.tile([128, 512], F32, tag="r", name=f"r_ps{p}")
                        kv = KV_HI[p] if hi else KV_LO[p]
                        cpos = 64 if hi else 0
                        nc.tensor.matmul(
                            o_banks[p][cpos:cpos + 64, :], V_sb[:, st, kv, :],
                            ep_[:, i_, :], start=(st == 0), stop=(st == ST - 1),
                            tile_position=(0, cpos), skip_group_check=True)
                        if st == ST - 1 and hi:
                            tails.append(p)
                    for i_, s in enumerate(grp):
                        p, st, hi = slot_info(s)
                        cpos = 64 if hi else 0
                        nc.tensor.matmul(
                            r_banks[p][cpos:cpos + 1, :], ones_bf,
                            ep_[:, i_, :], start=(st == 0), stop=(st == ST - 1),
                            tile_position=(0, cpos), skip_group_check=True)
                    for p in tails:
                        quad_tail(p)

                def quad_tail(p):
                    o_ps, r_ps = o_banks.pop(p), r_banks.pop(p)
                    r_st = tlp.tile([128, 512], BF16, tag="rst")
                    nc.vector.memset(r_st, 0.0)
                    nc.vector.tensor_copy(out=r_st[0:1, :], in_=r_ps[0:1, :])
                    nc.vector.tensor_copy(out=r_st[64:65, :], in_=r_ps[64:65, :])
                    o_st = tlp.tile([128, 512], BF16, tag="ost")
                    for k in range(4):
                        nc.vector.tensor_copy(out=o_st[:, 128 * k:128 * (k + 1)],
                                              in_=o_ps[:, 128 * k:128 * (k + 1)])
                    for k in range(4):
                        tt, ch = k % 2, k // 2
                        rt = tlp.tile([128, 128], BF16, tag="rt")
                        ot = tlp.tile([128, 128], BF16, tag="ot")
                        nc.scalar.dma_start(out=rt, in_=r_st[:, 128 * k:128 * (k + 1)],
                                            transpose=True)
                        nc.sync.dma_start(out=ot, in_=o_st[:, 128 * k:128 * (k + 1)],
                                          transpose=True)
                        rec = tlp.tile([128, 2], F32, tag="rec")
                        nc.vector.reciprocal(out=rec[:, 0:1], in_=rt[:, 0:1])
                        nc.vector.reciprocal(out=rec[:, 1:2], in_=rt[:, 64:65])
                        hE, hO = EH[2 * p + ch], OH[2 * p + ch]
                        nc.vector.tensor_scalar_mul(
                            out=OUT[:, tt, HD * hE:HD * (hE + 1)],
                            in0=ot[:, 0:64], scalar1=rec[:, 0:1])
                        nc.vector.tensor_scalar_mul(
                            out=OUT[:, tt, HD * hO:HD * (hO + 1)],
                            in0=ot[:, 64:128], scalar1=rec[:, 1:2])

                prev = None  # (grp, ep) awaiting O/R emission
                for gi, grp in enumerate(groups):
                    for s in grp:
                        ensure(*s_needs(s))
                    k = len(grp)
                    sp = (srA if k == 3 else srB).tile(
                        [128, k, 512], F32, tag="sA" if k == 3 else "sB")
                    ep_ = erp.tile([128, 3, 512], BF16, tag="e")
                    emit_s(grp, sp, ep_)
                    # keep draining fillers at ~1.3 sets per group
                    want = min(len(fillers), (4 * (gi + 1)) // 3)
                    while f_idx < want:
                        run_filler()
                    nc.scalar.activation(out=ep_[:, 0:k, :], in_=sp[:, 0:k, :],
                                         func=AF.Exp, scale=SCALE)
                    if prev is not None:
                        pgrp, pep = prev
                        ensure(*[("v", slot_info(s)[1]) for s in pgrp])
                        emit_or(pgrp, pep)
                    prev = (grp, ep_)
                # drain the final group
                pgrp, pep = prev
                ensure(*[("v", slot_info(s)[1]) for s in pgrp])
                emit_or(pgrp, pep)
                while f_idx < len(fillers):
                    run_filler()

            # ---------------- layernorm + store ----------------
            with tc.tile_pool(name="ln", bufs=2) as lnp:
                for tt in range(2):
                    stats = lnp.tile([128, 2, 6], F32, tag="stats")
                    nc.vector.bn_stats(out=stats[:, 0, :], in_=OUT[:, tt, 0:512])
                    nc.vector.bn_stats(out=stats[:, 1, :], in_=OUT[:, tt, 512:1024])
                    mv = lnp.tile([128, 2], F32, tag="mv")
                    nc.vector.bn_aggr(out=mv, in_=stats)
                    rstd = lnp.tile([128, 1], F32, tag="rstd")
                    nc.scalar.activation(out=rstd, in_=mv[:, 1:2], func=AF.Sqrt,
                                         bias=eps_t, scale=1.0)
                    nc.vector.reciprocal(out=rstd, in_=rstd)
                    y = lnp.tile([128, E], F32, tag="y")
                    nc.vector.tensor_scalar(out=y, in0=OUT[:, tt, :],
                                            scalar1=mv[:, 0:1], scalar2=rstd,
                                            op0=ALU.subtract, op1=ALU.mult)
                    z = lnp.tile([128, E], F32, tag="z")
                    nc.vector.tensor_mul(out=z, in0=y, in1=gamB)
                    nc.vector.tensor_add(out=z, in0=z, in1=betB)
                    nc.sync.dma_start(out=out_d[128 * tt:128 * (tt + 1), :], in_=z)

    nc.compile()
    return nc


def _prep_inputs(query, key, value, Wq, bq, Wk, bk, Wv, bv, gamma, beta):
    bf = ml_dtypes.bfloat16
    query, key, value = np.asarray(query), np.asarray(key), np.asarray(value)
    Wq, Wk, Wv = np.asarray(Wq), np.asarray(Wk), np.asarray(Wv)
    bq, bk, bv = np.asarray(bq), np.asarray(bk), np.asarray(bv)
    qT = np.ascontiguousarray(query[0].T.astype(bf))
    kT = np.ascontiguousarray(key[0].T.astype(bf))
    vT = np.ascontiguousarray(value[0].T.astype(bf))
    wqT = np.ascontiguousarray(Wq.T[:, COLPERM].astype(bf))
    wkT = np.ascontiguousarray(Wk.T.astype(bf))
    wvT = np.ascontiguousarray(Wv.T.astype(bf))
    bq_p = np.ascontiguousarray(bq[COLPERM].reshape(8, 128).T.astype(np.float32))
    bk_p = np.ascontiguousarray(bk.reshape(2, 128).T.astype(np.float32))
    common = {
        "kT": kT, "vT": vT, "wqT": wqT, "wkT": wkT, "wvT": wvT,
        "bq": bq_p, "bk": bk_p, "bv": np.asarray(bv, np.float32),
        "gam": np.asarray(gamma, np.float32), "bet": np.asarray(beta, np.float32),
    }
    in_maps = []
    for c in range(N_CORES):
        m = dict(common)
        m["qT"] = np.ascontiguousarray(qT[:, T * c:T * (c + 1)])
        in_maps.append(m)
    return in_maps


def run(inputs, trace=False):
    if "nc" not in _CACHE:
        _CACHE["nc"] = _build()
    nc = _CACHE["nc"]
    in_maps = _prep_inputs(**inputs)
    res = bass_utils.run_bass_kernel_spmd(
        nc, in_maps, core_ids=list(range(N_CORES)), trace=trace)
    out = np.empty((1, SEQ, E), np.float32)
    for c in range(N_CORES):
        out[0, T * c:T * (c + 1), :] = res.results[c]["out"]
    return out, res


def kernel(**inputs):
    out, _ = run(inputs, trace=False)
    return out
